# revision 4
# baseline (speedup 1.0000x reference)
"""Trainium2 Bass kernel for nn_DelayedMLP (B=8, S=2048, I=1024, H=4096, O=1024).

Sharding: data-parallel over batch — core b computes batch row b.

Per-core math (all on one NeuronCore, everything feature-major on chip):
  decayT[i,s]  = sigmoid(sum_j Wg[i,j] * xT[j,s] + bg[i])        (PE + ACT)
  immT         = xT * decayT                                      (DVE)
  delayedT     = xT - immT                                        (DVE)
  bufsT[:,t]   = bufsT[:,t-1] * decayT[:,t] + delayedT[:,t]       (DVE tensor_tensor_scan)
  combT        = immT + bufsT                                     (DVE)
  hidT[h,s]    = relu(sum_j W1[h,j] * combT[j,s] + b1[h])         (PE + ACT)
  out[s,o]     = sum_h hidT[h,s] * W2[o,h] + b2[o]                (PE, bias via K=1 ones-matmul)

Host side pre-transposes x/Wg/W1/W2 into the on-chip layouts and casts the
matmul operands to bf16 (fp32 accumulation in PSUM). Output stays fp32.
"""

import numpy as np
import ml_dtypes

import concourse.bass as bass
import concourse.mybir as mybir
import concourse.tile as tile
from concourse import bacc, bass_utils

P = 128
B, S, I, H, O = 8, 2048, 1024, 4096, 1024
KI = I // P           # 8 contraction subtiles over I
KH = H // P           # 32 contraction subtiles over H
C1 = 512              # phase-1 (gate/scan) sequence chunk
C2 = 512              # phase-2 (MLP) sequence chunk = mm1 moving free dim
OC = 512              # mm2 output free-dim chunk

BF16 = mybir.dt.bfloat16
F32 = mybir.dt.float32
AF = mybir.ActivationFunctionType
ALU = mybir.AluOpType
NP_BF16 = ml_dtypes.bfloat16


def build(nc: bass.Bass, S_: int = S):
    assert S_ % C1 == 0 and S_ % C2 == 0
    nch1 = S_ // C1
    nch2 = S_ // C2

    xT = nc.dram_tensor("xT", [I, S_], BF16, kind="ExternalInput").ap()
    wgT = nc.dram_tensor("WgT", [I, I], BF16, kind="ExternalInput").ap()
    w1T = nc.dram_tensor("W1T", [I, H], BF16, kind="ExternalInput").ap()
    w2T = nc.dram_tensor("W2T", [H, O], BF16, kind="ExternalInput").ap()
    bgT = nc.dram_tensor("bgT", [P, KI], F32, kind="ExternalInput").ap()
    b1T = nc.dram_tensor("b1T", [P, KH], F32, kind="ExternalInput").ap()
    b2r = nc.dram_tensor("b2r", [1, O], BF16, kind="ExternalInput").ap()
    out = nc.dram_tensor("out", [S_, O], F32, kind="ExternalOutput").ap()

    vx = xT.rearrange("(ko p) s -> p ko s", p=P)
    vwg = wgT.rearrange("(ko p) i -> p ko i", p=P)
    vw1 = w1T.rearrange("(ko p) h -> p ko h", p=P)
    vw2 = w2T.rearrange("(kh p) o -> p kh o", p=P)

    with tile.TileContext(nc) as tc:
        with tc.tile_pool(name="const", bufs=1) as cp:
            bg_sb = cp.tile([P, KI], F32, tag="bg")
            nc.sync.dma_start(bg_sb[:], bgT)
            b1_sb = cp.tile([P, KH], F32, tag="b1")
            nc.sync.dma_start(b1_sb[:], b1T)
            b2_sb = cp.tile([1, O], BF16, tag="b2")
            nc.sync.dma_start(b2_sb[:], b2r)
            ones_sb = cp.tile([1, P], BF16, tag="ones")
            nc.vector.memset(ones_sb[:], 1.0)

            with tc.tile_pool(name="comb", bufs=1) as combp:
                comb = combp.tile([P, KI, S_], BF16, tag="comb")

                # ---------------- phase 1: gate + scan ----------------
                with tc.tile_pool(name="wg", bufs=1) as wgp, \
                     tc.tile_pool(name="p1a", bufs=2) as p1a, \
                     tc.tile_pool(name="p1b", bufs=1) as p1b, \
                     tc.tile_pool(name="gps", bufs=2, space="PSUM") as gps:
                    wg_sb = []
                    for ko in range(KI):
                        t = wgp.tile([P, I], BF16, tag=f"wg{ko}")
                        nc.sync.dma_start(t[:], vwg[:, ko, :])
                        wg_sb.append(t)

                    prev_bufs = None
                    for c in range(nch1):
                        sl = slice(c * C1, (c + 1) * C1)
                        x_sb = p1a.tile([P, KI, C1], BF16, tag="x")
                        nc.sync.dma_start(x_sb[:], vx[:, :, sl])
                        dec = p1a.tile([P, KI, C1], BF16, tag="dec")
                        for it in range(KI):
                            ps = gps.tile([P, C1], F32, tag="g")
                            for ko in range(KI):
                                nc.tensor.matmul(
                                    ps[:], wg_sb[ko][:, it * P:(it + 1) * P],
                                    x_sb[:, ko, :],
                                    start=(ko == 0), stop=(ko == KI - 1))
                            nc.scalar.activation(dec[:, it, :], ps[:], AF.Sigmoid,
                                                 bias=bg_sb[:, it:it + 1])
                        imm = p1b.tile([P, KI, C1], BF16, tag="imm")
                        nc.vector.tensor_mul(imm[:], dec[:], x_sb[:])
                        dl = p1b.tile([P, KI, C1], BF16, tag="dl")
                        nc.vector.tensor_sub(dl[:], x_sb[:], imm[:])
                        bf = p1a.tile([P, KI, C1], BF16, tag="bufs")
                        for it in range(KI):
                            init = 0.0 if prev_bufs is None \
                                else prev_bufs[:, it, C1 - 1:C1]
                            nc.vector.tensor_tensor_scan(
                                bf[:, it, :], dec[:, it, :], dl[:, it, :], init,
                                op0=ALU.mult, op1=ALU.add)
                        prev_bufs = bf
                        nc.vector.tensor_add(comb[:, :, sl], imm[:], bf[:])

                # ---------------- phase 2: MLP ----------------
                with tc.tile_pool(name="w1", bufs=1) as w1p, \
                     tc.tile_pool(name="w2", bufs=1) as w2p, \
                     tc.tile_pool(name="hid", bufs=1) as hidp, \
                     tc.tile_pool(name="outp", bufs=4) as outp, \
                     tc.tile_pool(name="hps", bufs=2, space="PSUM") as hps, \
                     tc.tile_pool(name="ops", bufs=4, space="PSUM") as ops:
                    w1_sb = []
                    for ko in range(KI):
                        t = w1p.tile([P, H], BF16, tag=f"w1_{ko}")
                        nc.sync.dma_start(t[:], vw1[:, ko, :])
                        w1_sb.append(t)
                    w2_sb = []
                    for kh in range(KH):
                        t = w2p.tile([P, O], BF16, tag=f"w2_{kh}")
                        nc.sync.dma_start(t[:], vw2[:, kh, :])
                        w2_sb.append(t)

                    for c in range(nch2):
                        sl = slice(c * C2, (c + 1) * C2)
                        hid = hidp.tile([P, KH, C2], BF16, tag="hid")
                        for ht in range(KH):
                            ps = hps.tile([P, C2], F32, tag="h")
                            for ko in range(KI):
                                nc.tensor.matmul(
                                    ps[:], w1_sb[ko][:, ht * P:(ht + 1) * P],
                                    comb[:, ko, sl],
                                    start=(ko == 0), stop=(ko == KI - 1))
                            nc.scalar.activation(hid[:, ht, :], ps[:], AF.Relu,
                                                 bias=b1_sb[:, ht:ht + 1])
                        for ss in range(C2 // P):
                            r0 = c * C2 + ss * P
                            for oc in range(O // OC):
                                ps = ops.tile([P, OC], F32, tag="o")
                                nc.tensor.matmul(
                                    ps[:], ones_sb[:], b2_sb[:, oc * OC:(oc + 1) * OC],
                                    start=True, stop=False)
                                for kh in range(KH):
                                    nc.tensor.matmul(
                                        ps[:], hid[:, kh, ss * P:(ss + 1) * P],
                                        w2_sb[kh][:, oc * OC:(oc + 1) * OC],
                                        start=False, stop=(kh == KH - 1))
                                ot = outp.tile([P, OC], F32, tag="ot")
                                nc.vector.tensor_copy(ot[:], ps[:])
                                nc.sync.dma_start(
                                    out[r0:r0 + P, oc * OC:(oc + 1) * OC], ot[:])
    return nc


def make_nc(S_: int = S) -> bass.Bass:
    nc = bacc.Bacc("TRN2", target_bir_lowering=False, debug=False,
                   enable_asserts=False)
    build(nc, S_)
    nc.compile()
    return nc


def prep_in_maps(inputs: dict) -> list[dict]:
    x = np.asarray(inputs["x"], np.float32)
    Wg = np.asarray(inputs["Wg"], np.float32)
    W1 = np.asarray(inputs["W1"], np.float32)
    W2 = np.asarray(inputs["W2"], np.float32)
    bg = np.asarray(inputs["bg"], np.float32)
    b1 = np.asarray(inputs["b1"], np.float32)
    b2 = np.asarray(inputs["b2"], np.float32)

    shared = {
        "WgT": Wg.T.astype(NP_BF16),                      # [j, i]
        "W1T": W1.T.astype(NP_BF16),                      # [j, h]
        "W2T": W2.T.astype(NP_BF16),                      # [h, o]
        "bgT": np.ascontiguousarray(bg.reshape(KI, P).T), # [p, it]
        "b1T": np.ascontiguousarray(b1.reshape(KH, P).T), # [p, ht]
        "b2r": b2.astype(NP_BF16).reshape(1, O),
    }
    in_maps = []
    for b in range(B):
        m = dict(shared)
        m["xT"] = x[b].T.astype(NP_BF16)                  # [i, s]
        in_maps.append(m)
    return in_maps


LAST_RESULTS = None


def kernel(**inputs) -> np.ndarray:
    global LAST_RESULTS
    nc = make_nc()
    in_maps = prep_in_maps(inputs)
    res = bass_utils.run_bass_kernel_spmd(nc, in_maps, core_ids=list(range(B)))
    LAST_RESULTS = res
    out = np.stack([r["out"] for r in res.results], axis=0)
    return out.astype(np.float32)
